# revision 21
# baseline (speedup 1.0000x reference)
"""Trainium2 Bass kernel for nn_CrossDimensionalAttention_60550448939365.

Math reduction: in the reference, scores[b,i,j] = tp[b,i] . fp[b] is constant
in j, so softmax over j is exactly uniform (1/S) and attended[b,i,:] = fp[b,:].
The whole Wt/scores/softmax/bmm pipeline is a no-op. What remains:

    fp  = static @ Wf.T + bf                       # [B,H]
    z   = x + fp[b]                                # broadcast over seq
    xn  = normalize(z)            (LN1 core)
    y   = xn @ W2 + c2            where W2 = g1[:,None]*(Wo.T + I)  [h,k]
                                        c2 = b1 + bo + Wo @ b1
    out = normalize(y) * g2 + b2  (LN2)

W2/c2/fp are tiny host-side weight transforms; the device kernel does the
memory-heavy part: per core 1024 rows of [H=512]: LN1 -> transpose ->
matmul(W2) -> LN2 -> affine.

The program is specialized at build time on input properties detected on the
host (c2 == 0, g2 == 1, b2 == 0 -- which hold for this model's checkpoint);
a general program is built when they don't hold, so kernel() is correct for
any inputs.

Sharding: rows of flattened [B*S, H] = [8192, 512] split evenly across the 8
cores (1024 rows each, each shard entirely within one batch b = core//2).
"""

import os
import numpy as np

import concourse.bass as bass
import concourse.tile as tile
from concourse import bacc, mybir
from concourse.bass_utils import run_bass_kernel_spmd
from concourse.masks import make_identity

H = 512
B = 4
S = 2048
N_CORES = 8
ROWS = (B * S) // N_CORES  # 1024 rows per core
P = 128
NT = ROWS // P             # 8 token tiles per core
EPS = 1e-5

F32 = mybir.dt.float32
F32R = mybir.dt.float32r
AF = mybir.ActivationFunctionType
ALU = mybir.AluOpType

# "f32": exact fp32 matmuls (4 cycles/row on PE).
# "f32r": TF32 fast path (1 cycle/row, ~2e-4 rel err). Operands must be
#         produced (rounded) by compute engines, not DMA.
MATMUL_MODE = os.environ.get("KERNEL_MATMUL_MODE", "f32r")


def _bcast_ap(src: bass.AP, parts: int) -> bass.AP:
    """View a [N]-shaped DRAM AP as [parts, N] with 0-stride partitions."""
    return bass.AP(tensor=src.tensor, offset=src.offset, ap=[[0, parts]] + list(src.ap))


def _row_ap(src: bass.AP) -> bass.AP:
    """View a [N]-shaped DRAM AP as [1, N]."""
    return bass.AP(tensor=src.tensor, offset=src.offset, ap=[[0, 1]] + list(src.ap))


def build_program(with_c2: bool, with_affine2: bool) -> bass.Bass:
    nc = bacc.Bacc("TRN2", target_bir_lowering=False, debug=False)

    x = nc.dram_tensor("x", [ROWS, H], F32, kind="ExternalInput").ap()
    w2 = nc.dram_tensor("w2", [H, H], F32, kind="ExternalInput").ap()   # [h,k]
    c2 = nc.dram_tensor("c2", [H], F32, kind="ExternalInput").ap()
    fp = nc.dram_tensor("fp", [H], F32, kind="ExternalInput").ap()
    g2 = nc.dram_tensor("g2", [H], F32, kind="ExternalInput").ap()
    b2 = nc.dram_tensor("b2", [H], F32, kind="ExternalInput").ap()
    out = nc.dram_tensor("out", [ROWS, H], F32, kind="ExternalOutput").ap()

    MD = F32R if MATMUL_MODE == "f32r" else F32

    with tile.TileContext(nc) as tc:
        with (
            tc.tile_pool(name="consts", bufs=1) as consts,
            tc.tile_pool(name="xs", bufs=4) as xs,
            tc.tile_pool(name="zs", bufs=4) as zs,
            tc.tile_pool(name="xns", bufs=8) as xns,
            tc.tile_pool(name="xnts", bufs=3) as xnts,
            tc.tile_pool(name="stats", bufs=6) as stats,
            tc.tile_pool(name="smalls", bufs=12) as smalls,
            tc.tile_pool(name="ts", bufs=3) as ts_pool,
            tc.tile_pool(name="outs", bufs=3) as outs,
            tc.tile_pool(name="psum_t", bufs=3, space="PSUM") as psum_t,
            tc.tile_pool(name="psum_y", bufs=3, space="PSUM") as psum_y,
            tc.tile_pool(name="psum_d", bufs=1, space="PSUM") as psum_d,
        ):
            # ---- one-time constants ----
            fpb = consts.tile([P, H], F32)
            nc.gpsimd.dma_start(out=fpb, in_=_bcast_ap(fp, P))
            if with_affine2:
                g2b = consts.tile([P, H], F32)
                nc.gpsimd.dma_start(out=g2b, in_=_bcast_ap(g2, P))
                b2b = consts.tile([P, H], F32)
                nc.gpsimd.dma_start(out=b2b, in_=_bcast_ap(b2, P))

            w2s = consts.tile([P, 4, H], F32)  # [p, htile, k]; h = htile*128 + p
            nc.sync.dma_start(out=w2s, in_=w2.rearrange("(t p) k -> p t k", p=P))

            # PE matmul instructions only tolerate a single sync wait, so every
            # matmul operand must be produced by exactly one engine: W2 via an
            # ACT copy (same engine as the xnT evacuations), ones/c2 via DVE.
            w2mm = consts.tile([P, 4, H], MD)
            nc.scalar.copy(w2mm, w2s)
            if with_c2:
                c2row = consts.tile([1, H], F32)
                nc.sync.dma_start(out=c2row, in_=_row_ap(c2))
                c2mm = consts.tile([1, H], MD)
                nc.vector.tensor_copy(c2mm, c2row)
                ones1 = consts.tile([1, P], F32)
                nc.vector.memset(ones1, 1.0)
                onesmm = consts.tile([1, P], MD)
                nc.vector.tensor_copy(onesmm, ones1)

            iden_f32 = consts.tile([P, P], F32)
            make_identity(nc, iden_f32)
            if MD is F32R:
                iden = consts.tile([P, P], F32R)
                nc.gpsimd.tensor_copy(iden, iden_f32)
            else:
                iden = iden_f32
            epst = consts.tile([P, 1], F32)
            nc.vector.memset(epst, EPS)

            # Dummy PE ops that absorb the one-time cross-engine waits
            # (identity from GPSIMD, ones/c2 from DVE, w2 from ACT) so the
            # steady-state matmuls each need at most one sync wait.
            d1 = psum_d.tile([P, P], MD, tag="dummy1")
            nc.tensor.transpose(d1, iden, iden)
            d2 = psum_d.tile([P, P], F32, tag="dummy2")
            nc.tensor.matmul(d2, w2mm[:, 0, :P], w2mm[:, 0, :P], start=True, stop=True)
            if with_c2:
                nc.tensor.matmul(d2, onesmm, c2mm[:, :P], start=True, stop=True)

            # ---- software-pipelined main loop ----
            # stage A(i):   DMA + LN1 stats + xn        (DVE/ACT)
            # stage B(i-2): PE transposes + ACT evac
            # stage C(i-3): W2 matmuls + LN2 + store
            # Emitting B(i) before C(i-1) keeps the PE stream gap-free: by the
            # time the PE reaches C(i-1)'s matmuls, the evac is long done.
            xn_all, xnt_all, py_all = {}, {}, {}
            for i in range(NT + 3):
                if i < NT:
                    xt = xs.tile([P, H], F32)
                    nc.sync.dma_start(out=xt, in_=x[i * P:(i + 1) * P, :])

                    z = zs.tile([P, H], F32)
                    nc.vector.tensor_add(z, xt, fpb)

                    st1 = stats.tile([P, 6], F32, tag="st")
                    nc.vector.bn_stats(st1, z)
                    mv1 = stats.tile([P, 2], F32, tag="mv")
                    nc.vector.bn_aggr(mv1, st1)
                    sd1 = smalls.tile([P, 1], F32, tag="sd")
                    nc.scalar.activation(sd1, mv1[:, 1:2], AF.Sqrt, bias=epst,
                                         scale=1.0)
                    s1 = smalls.tile([P, 1], F32, tag="s")
                    nc.vector.reciprocal(s1, sd1)
                    negms1 = smalls.tile([P, 1], F32, tag="negms")
                    nc.vector.tensor_scalar(
                        negms1, mv1[:, 0:1], s1, -1.0, op0=ALU.mult, op1=ALU.mult
                    )
                    # xn = (z - m) * s on ACT; rounds to f32r for the PE.
                    xn = xns.tile([P, H], MD)
                    nc.scalar.activation(xn, z, AF.Identity, bias=negms1, scale=s1)
                    xn_all[i] = xn

                if 2 <= i < NT + 2:
                    j = i - 2
                    xn = xn_all[j]
                    ptr = psum_t.tile([P, 4, P], MD)
                    for h in range(4):
                        nc.tensor.transpose(ptr[:, h, :], xn[:, h * P:(h + 1) * P],
                                            iden)
                    xnt = xnts.tile([P, 4, P], MD)
                    nc.scalar.copy(xnt, ptr)
                    xnt_all[j] = xnt

                if i >= 3:
                    k = i - 3
                    xnt = xnt_all[k]
                    py = psum_y.tile([P, H], F32)
                    if with_c2:
                        nc.tensor.matmul(py, onesmm, c2mm, start=True, stop=False)
                    for h in range(4):
                        nc.tensor.matmul(
                            py, xnt[:, h, :], w2mm[:, h, :],
                            start=(h == 0 and not with_c2), stop=(h == 3),
                        )

                    st2 = stats.tile([P, 6], F32, tag="st")
                    nc.vector.bn_stats(st2, py)
                    mv2 = stats.tile([P, 2], F32, tag="mv")
                    nc.vector.bn_aggr(mv2, st2)
                    sd2 = smalls.tile([P, 1], F32, tag="sd")
                    nc.scalar.activation(sd2, mv2[:, 1:2], AF.Sqrt, bias=epst,
                                         scale=1.0)
                    s2 = smalls.tile([P, 1], F32, tag="s")
                    nc.vector.reciprocal(s2, sd2)
                    negms2 = smalls.tile([P, 1], F32, tag="negms")
                    nc.vector.tensor_scalar(
                        negms2, mv2[:, 0:1], s2, -1.0, op0=ALU.mult, op1=ALU.mult
                    )

                    t = ts_pool.tile([P, H], F32)
                    nc.scalar.activation(t, py, AF.Identity, bias=negms2, scale=s2)

                    if with_affine2:
                        t2 = outs.tile([P, H], F32, tag="t2")
                        nc.gpsimd.tensor_mul(t2, t, g2b)
                        ot = outs.tile([P, H], F32, tag="ot")
                        nc.gpsimd.tensor_add(ot, t2, b2b)
                    else:
                        ot = t

                    nc.sync.dma_start(out=out[k * P:(k + 1) * P, :], in_=ot)

    nc.compile()
    return nc


def _host_prep(temporal_features, static_features, Wt, bt, Wf, bf, Wo, bo,
               g1, b1, g2, b2):
    f32 = np.float32
    x = np.ascontiguousarray(np.asarray(temporal_features, dtype=f32)).reshape(B * S, H)
    st = np.asarray(static_features, dtype=f32)
    Wf = np.asarray(Wf, dtype=f32)
    bf = np.asarray(bf, dtype=f32)
    Wo = np.asarray(Wo, dtype=f32)
    bo = np.asarray(bo, dtype=f32)
    g1 = np.asarray(g1, dtype=f32)
    b1 = np.asarray(b1, dtype=f32)
    g2 = np.asarray(g2, dtype=f32)
    b2 = np.asarray(b2, dtype=f32)

    fp = st @ Wf.T + bf                                        # [B,H]
    W2 = g1[:, None] * (Wo.T + np.eye(H, dtype=f32))           # [h,k]
    c2 = b1 + bo + Wo @ b1                                     # [k]

    in_maps = []
    for c in range(N_CORES):
        shard = np.ascontiguousarray(x[c * ROWS:(c + 1) * ROWS])
        in_maps.append({
            "x": shard,
            "w2": np.ascontiguousarray(W2),
            "c2": np.ascontiguousarray(c2),
            "fp": np.ascontiguousarray(fp[(c * ROWS) // S]),
            "g2": np.ascontiguousarray(g2),
            "b2": np.ascontiguousarray(b2),
        })
    return in_maps


_NC_CACHE = {}


def _get_program(with_c2: bool, with_affine2: bool):
    key = (MATMUL_MODE, with_c2, with_affine2)
    if key not in _NC_CACHE:
        _NC_CACHE[key] = build_program(with_c2, with_affine2)
    return _NC_CACHE[key]


def run(inputs: dict, trace: bool = False):
    """Returns (output [B,S,H] f32, BassKernelResults)."""
    in_maps = _host_prep(**inputs)
    with_c2 = bool(np.any(in_maps[0]["c2"] != 0.0))
    with_affine2 = bool(
        np.any(in_maps[0]["g2"] != 1.0) or np.any(in_maps[0]["b2"] != 0.0)
    )
    nc = _get_program(with_c2, with_affine2)
    res = run_bass_kernel_spmd(nc, in_maps, list(range(N_CORES)), trace=trace)
    shards = [res.results[c]["out"] for c in range(N_CORES)]
    full = np.concatenate(shards, axis=0).reshape(B, S, H).astype(np.float32)
    return full, res


def kernel(**inputs) -> np.ndarray:
    out, _ = run(inputs, trace=False)
    return out


# revision 23
# speedup vs baseline: 1.0014x; 1.0014x over previous
"""Trainium2 Bass kernel for nn_CrossDimensionalAttention_60550448939365.

Math reduction: in the reference, scores[b,i,j] = tp[b,i] . fp[b] is constant
in j, so softmax over j is exactly uniform (1/S) and attended[b,i,:] = fp[b,:].
The whole Wt/scores/softmax/bmm pipeline is a no-op. What remains:

    fp  = static @ Wf.T + bf                       # [B,H]
    z   = x + fp[b]                                # broadcast over seq
    xn  = normalize(z)            (LN1 core)
    y   = xn @ W2 + c2            where W2 = g1[:,None]*(Wo.T + I)  [h,k]
                                        c2 = b1 + bo + Wo @ b1
    out = normalize(y) * g2 + b2  (LN2)

W2/c2/fp are tiny host-side weight transforms; the device kernel does the
memory-heavy part: per core 1024 rows of [H=512]: LN1 -> transpose ->
matmul(W2) -> LN2 -> affine.

The program is specialized at build time on input properties detected on the
host (c2 == 0, g2 == 1, b2 == 0 -- which hold for this model's checkpoint);
a general program is built when they don't hold, so kernel() is correct for
any inputs.

Sharding: rows of flattened [B*S, H] = [8192, 512] split evenly across the 8
cores (1024 rows each, each shard entirely within one batch b = core//2).
"""

import os
import numpy as np

import concourse.bass as bass
import concourse.tile as tile
from concourse import bacc, mybir
from concourse.bass_utils import run_bass_kernel_spmd
from concourse.masks import make_identity

H = 512
B = 4
S = 2048
N_CORES = 8
ROWS = (B * S) // N_CORES  # 1024 rows per core
P = 128
NT = ROWS // P             # 8 token tiles per core
EPS = 1e-5

F32 = mybir.dt.float32
F32R = mybir.dt.float32r
AF = mybir.ActivationFunctionType
ALU = mybir.AluOpType

# "f32": exact fp32 matmuls (4 cycles/row on PE).
# "f32r": TF32 fast path (1 cycle/row, ~2e-4 rel err). Operands must be
#         produced (rounded) by compute engines, not DMA.
MATMUL_MODE = os.environ.get("KERNEL_MATMUL_MODE", "f32r")


def _bcast_ap(src: bass.AP, parts: int) -> bass.AP:
    """View a [N]-shaped DRAM AP as [parts, N] with 0-stride partitions."""
    return bass.AP(tensor=src.tensor, offset=src.offset, ap=[[0, parts]] + list(src.ap))


def _row_ap(src: bass.AP) -> bass.AP:
    """View a [N]-shaped DRAM AP as [1, N]."""
    return bass.AP(tensor=src.tensor, offset=src.offset, ap=[[0, 1]] + list(src.ap))


def build_program(with_c2: bool, with_affine2: bool) -> bass.Bass:
    nc = bacc.Bacc("TRN2", target_bir_lowering=False, debug=False)

    x = nc.dram_tensor("x", [ROWS, H], F32, kind="ExternalInput").ap()
    w2 = nc.dram_tensor("w2", [H, H], F32, kind="ExternalInput").ap()   # [h,k]
    c2 = nc.dram_tensor("c2", [H], F32, kind="ExternalInput").ap()
    fp = nc.dram_tensor("fp", [H], F32, kind="ExternalInput").ap()
    g2 = nc.dram_tensor("g2", [H], F32, kind="ExternalInput").ap()
    b2 = nc.dram_tensor("b2", [H], F32, kind="ExternalInput").ap()
    out = nc.dram_tensor("out", [ROWS, H], F32, kind="ExternalOutput").ap()

    MD = F32R if MATMUL_MODE == "f32r" else F32

    with tile.TileContext(nc) as tc:
        with (
            tc.tile_pool(name="consts", bufs=1) as consts,
            tc.tile_pool(name="xs", bufs=4) as xs,
            tc.tile_pool(name="zs", bufs=4) as zs,
            tc.tile_pool(name="xns", bufs=8) as xns,
            tc.tile_pool(name="xnts", bufs=3) as xnts,
            tc.tile_pool(name="stats", bufs=6) as stats,
            tc.tile_pool(name="smalls", bufs=12) as smalls,
            tc.tile_pool(name="ts", bufs=3) as ts_pool,
            tc.tile_pool(name="outs", bufs=3) as outs,
            tc.tile_pool(name="psum_t", bufs=3, space="PSUM") as psum_t,
            tc.tile_pool(name="psum_y", bufs=3, space="PSUM") as psum_y,
            tc.tile_pool(name="psum_d", bufs=1, space="PSUM") as psum_d,
        ):
            # ---- one-time constants ----
            # Broadcasts ([H] -> [128,H]) go through a K=1 PE matmul + ACT
            # copy: the GPSIMD 0-stride DMA broadcast takes ~5us and
            # serializes the whole preamble.
            ones1 = consts.tile([1, P], F32)
            nc.vector.memset(ones1, 1.0)
            onesmm = consts.tile([1, P], MD)
            nc.vector.tensor_copy(onesmm, ones1)

            fprow = consts.tile([1, H], F32)
            nc.sync.dma_start(out=fprow, in_=_row_ap(fp))
            fpmm = consts.tile([1, H], MD)
            nc.vector.tensor_copy(fpmm, fprow)
            fp_ps = psum_d.tile([P, H], F32, tag="bcast")
            nc.tensor.matmul(fp_ps, onesmm, fpmm, start=True, stop=True)
            fpb = consts.tile([P, H], F32)
            nc.scalar.copy(fpb, fp_ps)

            if with_affine2:
                g2b = consts.tile([P, H], F32)
                nc.gpsimd.dma_start(out=g2b, in_=_bcast_ap(g2, P))
                b2b = consts.tile([P, H], F32)
                nc.gpsimd.dma_start(out=b2b, in_=_bcast_ap(b2, P))

            if with_c2:
                c2row = consts.tile([1, H], F32)
                nc.sync.dma_start(out=c2row, in_=_row_ap(c2))
                c2mm = consts.tile([1, H], MD)
                nc.vector.tensor_copy(c2mm, c2row)

            iden_f32 = consts.tile([P, P], F32)
            make_identity(nc, iden_f32)
            if MD is F32R:
                iden = consts.tile([P, P], F32R)
                nc.gpsimd.tensor_copy(iden, iden_f32)
            else:
                iden = iden_f32
            epst = consts.tile([P, 1], F32)
            nc.vector.memset(epst, EPS)

            # Dummy PE op absorbing the identity's GPSIMD wait.
            d1 = psum_d.tile([P, P], MD, tag="dummy")
            nc.tensor.transpose(d1, iden, iden)

            # ---- software-pipelined main loop ----
            # stage A(i):   DMA + LN1 stats + xn        (DVE/ACT)
            # stage B(i-2): PE transposes + ACT evac
            # stage C(i-3): W2 matmuls + LN2 + store
            # Emitting B(i) before C(i-1) keeps the PE stream gap-free: by the
            # time the PE reaches C(i-1)'s matmuls, the evac is long done.
            xn_all, xnt_all, py_all = {}, {}, {}
            w2mm = consts.tile([P, 4, H], MD)
            for i in range(NT + 3):
                if i == 1:
                    # W2 load emitted after tile 0's x-load so the first
                    # LN1 isn't stuck behind 1MB on the same DMA queue.
                    w2s = consts.tile([P, 4, H], F32)
                    nc.sync.dma_start(
                        out=w2s, in_=w2.rearrange("(t p) k -> p t k", p=P)
                    )
                    nc.scalar.copy(w2mm, w2s)

                if i < NT:
                    xt = xs.tile([P, H], F32)
                    nc.sync.dma_start(out=xt, in_=x[i * P:(i + 1) * P, :])

                    z = zs.tile([P, H], F32)
                    nc.vector.tensor_add(z, xt, fpb)

                    st1 = stats.tile([P, 6], F32, tag="st")
                    nc.vector.bn_stats(st1, z)
                    mv1 = stats.tile([P, 2], F32, tag="mv")
                    nc.vector.bn_aggr(mv1, st1)
                    sd1 = smalls.tile([P, 1], F32, tag="sd")
                    nc.scalar.activation(sd1, mv1[:, 1:2], AF.Sqrt, bias=epst,
                                         scale=1.0)
                    s1 = smalls.tile([P, 1], F32, tag="s")
                    nc.vector.reciprocal(s1, sd1)
                    negms1 = smalls.tile([P, 1], F32, tag="negms")
                    nc.vector.tensor_scalar(
                        negms1, mv1[:, 0:1], s1, -1.0, op0=ALU.mult, op1=ALU.mult
                    )
                    # xn = (z - m) * s on ACT; rounds to f32r for the PE.
                    xn = xns.tile([P, H], MD)
                    nc.scalar.activation(xn, z, AF.Identity, bias=negms1, scale=s1)
                    xn_all[i] = xn

                if 2 <= i < NT + 2:
                    j = i - 2
                    xn = xn_all[j]
                    ptr = psum_t.tile([P, 4, P], MD)
                    for h in range(4):
                        nc.tensor.transpose(ptr[:, h, :], xn[:, h * P:(h + 1) * P],
                                            iden)
                    xnt = xnts.tile([P, 4, P], MD)
                    nc.scalar.copy(xnt, ptr)
                    xnt_all[j] = xnt

                if i >= 3:
                    k = i - 3
                    xnt = xnt_all[k]
                    py = psum_y.tile([P, H], F32)
                    if with_c2:
                        nc.tensor.matmul(py, onesmm, c2mm, start=True, stop=False)
                    for h in range(4):
                        nc.tensor.matmul(
                            py, xnt[:, h, :], w2mm[:, h, :],
                            start=(h == 0 and not with_c2), stop=(h == 3),
                        )

                    st2 = stats.tile([P, 6], F32, tag="st")
                    nc.vector.bn_stats(st2, py)
                    mv2 = stats.tile([P, 2], F32, tag="mv")
                    nc.vector.bn_aggr(mv2, st2)
                    sd2 = smalls.tile([P, 1], F32, tag="sd")
                    nc.scalar.activation(sd2, mv2[:, 1:2], AF.Sqrt, bias=epst,
                                         scale=1.0)
                    s2 = smalls.tile([P, 1], F32, tag="s")
                    nc.vector.reciprocal(s2, sd2)
                    negms2 = smalls.tile([P, 1], F32, tag="negms")
                    nc.vector.tensor_scalar(
                        negms2, mv2[:, 0:1], s2, -1.0, op0=ALU.mult, op1=ALU.mult
                    )

                    t = ts_pool.tile([P, H], F32)
                    nc.scalar.activation(t, py, AF.Identity, bias=negms2, scale=s2)

                    if with_affine2:
                        t2 = outs.tile([P, H], F32, tag="t2")
                        nc.gpsimd.tensor_mul(t2, t, g2b)
                        ot = outs.tile([P, H], F32, tag="ot")
                        nc.gpsimd.tensor_add(ot, t2, b2b)
                    else:
                        ot = t

                    nc.sync.dma_start(out=out[k * P:(k + 1) * P, :], in_=ot)

    nc.compile()
    return nc


def _host_prep(temporal_features, static_features, Wt, bt, Wf, bf, Wo, bo,
               g1, b1, g2, b2):
    f32 = np.float32
    x = np.ascontiguousarray(np.asarray(temporal_features, dtype=f32)).reshape(B * S, H)
    st = np.asarray(static_features, dtype=f32)
    Wf = np.asarray(Wf, dtype=f32)
    bf = np.asarray(bf, dtype=f32)
    Wo = np.asarray(Wo, dtype=f32)
    bo = np.asarray(bo, dtype=f32)
    g1 = np.asarray(g1, dtype=f32)
    b1 = np.asarray(b1, dtype=f32)
    g2 = np.asarray(g2, dtype=f32)
    b2 = np.asarray(b2, dtype=f32)

    fp = st @ Wf.T + bf                                        # [B,H]
    W2 = g1[:, None] * (Wo.T + np.eye(H, dtype=f32))           # [h,k]
    c2 = b1 + bo + Wo @ b1                                     # [k]

    in_maps = []
    for c in range(N_CORES):
        shard = np.ascontiguousarray(x[c * ROWS:(c + 1) * ROWS])
        in_maps.append({
            "x": shard,
            "w2": np.ascontiguousarray(W2),
            "c2": np.ascontiguousarray(c2),
            "fp": np.ascontiguousarray(fp[(c * ROWS) // S]),
            "g2": np.ascontiguousarray(g2),
            "b2": np.ascontiguousarray(b2),
        })
    return in_maps


_NC_CACHE = {}


def _get_program(with_c2: bool, with_affine2: bool):
    key = (MATMUL_MODE, with_c2, with_affine2)
    if key not in _NC_CACHE:
        _NC_CACHE[key] = build_program(with_c2, with_affine2)
    return _NC_CACHE[key]


def run(inputs: dict, trace: bool = False):
    """Returns (output [B,S,H] f32, BassKernelResults)."""
    in_maps = _host_prep(**inputs)
    with_c2 = bool(np.any(in_maps[0]["c2"] != 0.0))
    with_affine2 = bool(
        np.any(in_maps[0]["g2"] != 1.0) or np.any(in_maps[0]["b2"] != 0.0)
    )
    nc = _get_program(with_c2, with_affine2)
    res = run_bass_kernel_spmd(nc, in_maps, list(range(N_CORES)), trace=trace)
    shards = [res.results[c]["out"] for c in range(N_CORES)]
    full = np.concatenate(shards, axis=0).reshape(B, S, H).astype(np.float32)
    return full, res


def kernel(**inputs) -> np.ndarray:
    out, _ = run(inputs, trace=False)
    return out
